# revision 37
# baseline (speedup 1.0000x reference)
"""Trainium2 Bass kernel for nn_MultiHeadSelfAttentionBlock.

Reference computation (per batch b):
    xn = LN(x; ln0)                    # layernorm over d=256
    Q  = xn @ Wq + bq   (2048, 256)    # 8 heads x 32
    K  = y  @ Wk + bk
    V  = y  @ Wv + bv
    S  = Q K^T / 16     per head       # (2048, 2048) x 8
    A  = softmax(S, axis=k)
    O  = Q + A V        per head
    On = LN(O; ln1)
    out = O + gelu(On @ W1 + b1) @ W2 + b2

Sharding: pure data parallel, one batch per NeuronCore (B=8, 8 cores).
No collectives.

Device-side layout: everything is kept feature-major ("transposed",
features on SBUF partitions, tokens on the free dim). The host passes
x^T / y^T and transposes the output back; LN gains/biases are folded
into Wq/W1 on the host (xn@Wq = z@(g*Wq) + (ln0_b@Wq + bq), z the
unscaled normalized x).

Matmuls run in fp32r (full PE rate at free dim >= 256). fp32r operands
must come from a rounding producer, so DMA-loaded operands are declared
float32r in DRAM and engine-produced operands are written to float32r
tiles.

Attention per head pair (2 heads), per 512-token q chunk, per 128-token
k block:
  - S^T block (128k x 512q per head) via one matmul each with 32-deep
    contraction, row-tiled into the PE array at partition 32*h.
  - exp on ScalarE directly PSUM->SBUF with the 1/16 scale folded in
    (scores are bounded, max-subtraction unnecessary: |S/16| < ~3).
  - A V and the softmax denominators via four M=32 col-tiled matmuls
    (stationary V_h / all-ones), K=128, accumulated over k blocks into
    one PSUM bank: rows 0-31 O_even, 32-63 denom_even, 64-95 O_odd,
    96-127 denom_odd.  All run with start=False over a zeroed bank
    (per-element has_written semantics).
  - epilogue: denominators are copied out, partition-shifted onto the O
    rows with identity matmuls (walrus requires identical partition
    ranges on all DVE operands), reciprocal'd, and multiplied in;
    a partition-remapping SBUF->SBUF DMA restores the natural feature
    order per pair, then one add applies the Q residual.
"""

import contextlib

import ml_dtypes
import numpy as np

import concourse.bass as bass
import concourse.mybir as mybir
import concourse.tile as tile
from concourse import bacc
from concourse.bass_utils import run_bass_kernel_spmd

F32 = mybir.dt.float32
F32R = mybir.dt.float32r
BF16 = mybir.dt.bfloat16
AF = mybir.ActivationFunctionType
ALU = mybir.AluOpType

B, N, D, DV, DFF, H, HD = 8, 2048, 256, 256, 512, 8, 32
EPS = 1e-5
CH = 512              # free-dim chunk for matmuls
NCH = N // CH         # 4
KB = N // 128         # 16 key blocks
NPAIR = H // 2        # 4 head pairs

_NC_CACHE: dict = {}
DEBUG_TAPS: set = set()   # names of intermediates to expose as outputs


def _tap(nc, name, ap):
    if name in DEBUG_TAPS:
        dt = F32 if ap.dtype in (F32, F32R) else ap.dtype
        d = nc.dram_tensor(f"tap_{name}", list(ap.shape), dt,
                           kind="ExternalOutput").ap()
        nc.sync.dma_start(out=d, in_=ap.bitcast(dt) if ap.dtype != dt else ap)


def _f(ap):
    """View an f32r AP as f32 for vector/scalar engine reads."""
    return ap.bitcast(F32) if ap.dtype == F32R else ap


def _layernorm_featmajor(nc, tc, work, cones, eps_ap, src, dst, n_feat, prefix):
    """LN over the partition (feature) dim of feature-major tiles.

    src: list of (128, N) f32r SBUF tiles covering n_feat features.
    dst: list of (128, N) f32r tiles receiving (src - mean) * rstd.
    Stats via ones-matmul (sum over partitions, broadcast to all 128
    output partitions); rstd = exp(-0.5 * ln(var + eps)) so everything
    stays in the natural_log_exp ACT table set.
    """
    nk = len(src)
    assert n_feat == 128 * nk
    with tc.tile_pool(name=f"{prefix}_psum", bufs=1, space="PSUM") as stats:
        s1b = stats.tile([128, N], F32, tag="stats", name=f"{prefix}_s1")
        for k in range(nk):
            for c in range(NCH):
                nc.tensor.matmul(
                    s1b[:, c * CH:(c + 1) * CH],
                    cones[:, :],
                    src[k][:, c * CH:(c + 1) * CH],
                    start=(k == 0), stop=(k == nk - 1),
                )
        mu = work.tile([128, N], F32, tag="ln_mu", name=f"{prefix}_mu")
        nc.vector.tensor_scalar_mul(mu[:, :], s1b[:, :], 1.0 / n_feat)

        s2b = stats.tile([128, N], F32, tag="stats", name=f"{prefix}_s2")
        sqs = []
        for k in range(nk):
            sq = work.tile([128, N], F32R, tag="ln_tmp", name=f"{prefix}_sq{k}")
            nc.vector.tensor_mul(sq[:, :], _f(src[k][:, :]), _f(src[k][:, :]))
            sqs.append(sq)
        for k in range(nk):
            for c in range(NCH):
                nc.tensor.matmul(
                    s2b[:, c * CH:(c + 1) * CH],
                    cones[:, :],
                    sqs[k][:, c * CH:(c + 1) * CH],
                    start=(k == 0), stop=(k == nk - 1),
                )
        musq = work.tile([128, N], F32, tag="ln_tmp", name=f"{prefix}_musq")
        nc.vector.tensor_mul(musq[:, :], mu[:, :], mu[:, :])
        var = work.tile([128, N], F32, tag="ln_tmp", name=f"{prefix}_var")
        nc.vector.scalar_tensor_tensor(
            var[:, :], s2b[:, :], 1.0 / n_feat, musq[:, :],
            op0=ALU.mult, op1=ALU.subtract,
        )
        # rstd = exp(-0.5 * ln(var + eps))
        lnv = work.tile([128, N], F32, tag="ln_tmp", name=f"{prefix}_lnv")
        nc.scalar.activation(lnv[:, :], var[:, :], AF.Ln, bias=eps_ap)
        rstd = work.tile([128, N], F32, tag="ln_rstd", name=f"{prefix}_rstd")
        nc.scalar.activation(rstd[:, :], lnv[:, :], AF.Exp, scale=-0.5)

        for k in range(nk):
            cen = work.tile([128, N], F32, tag="ln_tmp", name=f"{prefix}_cen{k}")
            nc.vector.tensor_sub(cen[:, :], _f(src[k][:, :]), mu[:, :])
            nc.vector.tensor_mul(dst[k][:, :], cen[:, :], rstd[:, :])


def build_device_kernel(nc, tc):
    ctx = contextlib.ExitStack()
    with ctx:
        # ---------------- DRAM params ----------------
        # Tensors that feed the PE are declared float32r (host passes raw
        # f32 bits; the PE's internal rounding applies either way).
        xT_d = nc.dram_tensor("xT", [D, N], F32R, kind="ExternalInput").ap()
        yT_d = nc.dram_tensor("yT", [D, N], F32R, kind="ExternalInput").ap()
        Wq_d = nc.dram_tensor("Wq", [D, DV], F32R, kind="ExternalInput").ap()
        Wk_d = nc.dram_tensor("Wk", [D, DV], F32R, kind="ExternalInput").ap()
        Wv_d = nc.dram_tensor("Wv", [D, DV], F32R, kind="ExternalInput").ap()
        W1_d = nc.dram_tensor("W1", [DV, DFF], F32R, kind="ExternalInput").ap()
        W2_d = nc.dram_tensor("W2", [DFF, DV], F32R, kind="ExternalInput").ap()
        cones_d = nc.dram_tensor("cones", [128, 128], F32R, kind="ExternalInput").ap()
        conesb_d = nc.dram_tensor("conesb", [128, 32], BF16, kind="ExternalInput").ap()
        bv_d = nc.dram_tensor("bv", [1, DV], F32R, kind="ExternalInput").ap()
        bq_d = nc.dram_tensor("bq", [DV], F32, kind="ExternalInput").ap()
        bk_d = nc.dram_tensor("bk", [DV], F32, kind="ExternalInput").ap()
        b1_d = nc.dram_tensor("b1", [DFF], F32, kind="ExternalInput").ap()
        b2_d = nc.dram_tensor("b2", [DV], F32, kind="ExternalInput").ap()
        outT_d = nc.dram_tensor("outT", [DV, N], F32, kind="ExternalOutput").ap()

        # ---------------- SBUF pools (whole-kernel lifetime) ----------------
        persist = ctx.enter_context(tc.tile_pool(name="persist", bufs=1))
        zn_pool = ctx.enter_context(tc.tile_pool(name="znp", bufs=2))
        xy_pool = ctx.enter_context(tc.tile_pool(name="xyp", bufs=4))
        work = ctx.enter_context(tc.tile_pool(name="work", bufs=2))
        at_pool = ctx.enter_context(tc.tile_pool(name="atp", bufs=2))
        sm_pool = ctx.enter_context(tc.tile_pool(name="smp", bufs=1))

        # ---------------- constants / weights ----------------
        cones = persist.tile([128, 128], F32R, tag="cones")
        nc.sync.dma_start(out=cones[:, :], in_=cones_d)
        conesb = persist.tile([128, 32], BF16, tag="conesb")
        nc.sync.dma_start(out=conesb[:, :], in_=conesb_d)
        bv_row = persist.tile([1, DV], F32R, tag="bvrow")
        nc.sync.dma_start(out=bv_row[:, :], in_=bv_d)
        eps_sb = persist.tile([128, 1], F32, tag="eps")
        nc.gpsimd.memset(eps_sb[:, :], EPS)
        eps_ap = eps_sb[:, 0:1]

        def load2(dram, cols, tagbase):
            ts = []
            for k in range(dram.shape[0] // 128):
                t = persist.tile([128, cols], F32R, tag=f"{tagbase}{k}",
                                 name=f"{tagbase}{k}")
                nc.sync.dma_start(out=t[:, :], in_=dram[k * 128:(k + 1) * 128, :])
                ts.append(t)
            return ts

        Wq_sb = load2(Wq_d, DV, "wq")
        Wk_sb = load2(Wk_d, DV, "wk")
        Wv_sb = load2(Wv_d, DV, "wv")
        W1_sb = load2(W1_d, DFF, "w1")
        W2_sb = load2(W2_d, DV, "w2")

        def loadbias(dram, n, tag):
            t = persist.tile([128, n // 128], F32, tag=tag, name=tag)
            nc.sync.dma_start(out=t[:, :], in_=dram.rearrange("(c p) -> p c", p=128))
            return t

        bq_sb = loadbias(bq_d, DV, "bq")
        bk_sb = loadbias(bk_d, DV, "bk")
        b1_sb = loadbias(b1_d, DFF, "b1")
        b2_sb = loadbias(b2_d, DV, "b2")

        # ---------------- load x^T, y^T ----------------
        xT = []
        for k in range(2):
            t = xy_pool.tile([128, N], F32R, tag="xy", name=f"xT{k}")
            nc.sync.dma_start(out=t[:, :], in_=xT_d[k * 128:(k + 1) * 128, :])
            xT.append(t)
        yT = []
        for k in range(2):
            t = xy_pool.tile([128, N], F32R, tag="xy", name=f"yT{k}")
            nc.sync.dma_start(out=t[:, :], in_=yT_d[k * 128:(k + 1) * 128, :])
            yT.append(t)

        # ---------------- LN0 on x^T ----------------
        znT = [zn_pool.tile([128, N], F32R, tag="zn", name=f"znT{k}")
               for k in range(2)]
        _layernorm_featmajor(nc, tc, work, cones, eps_ap, xT, znT, D, "ln0")
        _tap(nc, "znT0", znT[0][:, :])

        # ------- projections Q^T, K^T and V (phase-scoped PSUM) -------
        QT = [persist.tile([128, N], F32R, tag=f"qt{t}", name=f"QT{t}")
              for t in range(2)]
        KT = [persist.tile([128, N], F32R, tag=f"kt{t}", name=f"KT{t}")
              for t in range(2)]
        Vtm = [persist.tile([128, DV], BF16, tag=f"vtm{tb}", name=f"Vtm{tb}")
               for tb in range(KB)]
        with tc.tile_pool(name="ps_proj", bufs=2, space="PSUM") as ps_proj:
            for dst, w_sb, b_sb, src in ((KT, Wk_sb, bk_sb, yT), (QT, Wq_sb, bq_sb, znT)):
                for t in range(2):          # dv m-block
                    for c in range(NCH):    # token chunk
                        ps = ps_proj.tile([128, CH], F32, tag="proj", name="proj_ps")
                        for k in range(2):  # d_in contraction
                            nc.tensor.matmul(
                                ps[:, :],
                                w_sb[k][:, t * 128:(t + 1) * 128],
                                src[k][:, c * CH:(c + 1) * CH],
                                start=(k == 0), stop=(k == 1),
                            )
                        nc.vector.tensor_scalar_add(
                            dst[t][:, c * CH:(c + 1) * CH], ps[:, :], b_sb[:, t:t + 1],
                        )

            for tb in range(KB):
                ps = ps_proj.tile([128, DV], F32, tag="vproj", name="vproj_ps")
                for k in range(2):
                    nc.tensor.matmul(
                        ps[:, :],
                        yT[k][:, tb * 128:(tb + 1) * 128],
                        Wv_sb[k][:, :],
                        start=(k == 0), stop=False,
                    )
                nc.tensor.matmul(
                    ps[:, :], cones[0:1, :], bv_row[:, :],
                    start=False, stop=True,
                )
                nc.vector.tensor_copy(out=Vtm[tb][:, :], in_=ps[:, :])
        _tap(nc, "QT0", QT[0][:, :])
        _tap(nc, "KT0", KT[0][:, :])
        _tap(nc, "Vtm0", Vtm[0][:, :])

        # ---------------- attention ----------------
        # Pair-local result layout in opair (one PSUM bank):
        #   rows 0-31 O_even, 32-63 denom_even, 64-95 O_odd, 96-127 denom_odd
        # opl collects O/denom-normalized values at rows {0-31, 64-95}; the
        # pair DMA restores natural feature order into OTpre.
        OT = [persist.tile([128, N], F32R, tag=f"ot{t}", name=f"OT{t}")
              for t in range(2)]
        OTpre = [zn_pool.tile([128, N], F32, tag="zn", name=f"OTpre{t}")
                 for t in range(2)]
        with (tc.tile_pool(name="ps_s", bufs=2, space="PSUM") as ps_s,
              tc.tile_pool(name="ps_o", bufs=2, space="PSUM") as ps_o):
            for p in range(NPAIR):
                tK = p // 2               # QT/KT tile index
                base = 64 * (p % 2)       # partition base of this pair's features
                h0 = 2 * p
                opl = xy_pool.tile([128, N], F32, tag="xy", name=f"Opl{p}")
                for q in range(NCH):
                    qs = slice(q * CH, (q + 1) * CH)
                    opair = ps_o.tile([128, CH], F32, tag="opair", name="opair")
                    nc.vector.memset(opair[:, :], 0.0)
                    for kb in range(KB):
                        s = ps_s.tile([128, 1024], F32, tag="sblk", name="sblk")
                        for j in range(2):
                            rg = base + 32 * j
                            nc.tensor.matmul(
                                s[:, j * CH:(j + 1) * CH],
                                KT[tK][rg:rg + 32, kb * 128:(kb + 1) * 128],
                                QT[tK][rg:rg + 32, qs],
                                start=True, stop=True,
                                tile_position=(rg, 0),
                            )
                        at = at_pool.tile([128, 1024], BF16, tag="at", name="at")
                        nc.scalar.activation(at[:, :], s[:, :], AF.Exp, scale=1.0 / 16.0)
                        if p == 0 and q == 0 and kb == 0:
                            _tap(nc, "at00", at[:, :])
                        for j in range(2):
                            # O_h accumulation (V stationary, M=32)
                            nc.tensor.matmul(
                                opair[64 * j:64 * j + 32, :],
                                Vtm[kb][:, (h0 + j) * 32:(h0 + j + 1) * 32],
                                at[:, j * CH:(j + 1) * CH],
                                start=False, stop=False,
                                skip_group_check=True,
                                tile_position=(0, 64 * j),
                            )
                            # denominator accumulation (ones stationary, M=32)
                            nc.tensor.matmul(
                                opair[64 * j + 32:64 * j + 64, :],
                                conesb[:, :],
                                at[:, j * CH:(j + 1) * CH],
                                start=False, stop=False,
                                skip_group_check=True,
                                tile_position=(0, 64 * j + 32),
                            )
                    # epilogue: O_pair_local = O_unnorm * recip(denom).
                    # Denominators sit at rows {32-63, 96-127}. Walrus needs
                    # identical partition ranges on DVE operands and the
                    # custom-DVE reciprocal only works at partition base 0,
                    # so: copy denominators out of PSUM, DMA both heads' rows
                    # to rows 0-31, reciprocal there, DMA the odd head's
                    # reciprocals to rows 64-95, then multiply in place.
                    dcp = sm_pool.tile([128, CH], F32, tag="dcp", name="dcp")
                    rsrc = sm_pool.tile([32, 1024], F32, tag="rsrc", name="rsrc")
                    rec2 = sm_pool.tile([128, 1024], F32, tag="rec2", name="rec2")
                    for j in range(2):
                        db = 64 * j + 32
                        js = slice(j * CH, (j + 1) * CH)
                        nc.vector.tensor_copy(
                            out=dcp[db:db + 32, :], in_=opair[db:db + 32, :])
                        nc.sync.dma_start(
                            out=rsrc[0:32, js], in_=dcp[db:db + 32, :])
                    nc.vector.reciprocal_approx_fast(
                        rec2[0:32, :], rsrc[0:32, :])
                    nc.sync.dma_start(
                        out=rec2[64:96, CH:2 * CH], in_=rec2[0:32, CH:2 * CH])
                    for j in range(2):
                        ob = 64 * j
                        js = slice(j * CH, (j + 1) * CH)
                        nc.vector.tensor_mul(
                            opl[ob:ob + 32, qs], opair[ob:ob + 32, :],
                            rec2[ob:ob + 32, js])
                    if p == 0 and q == 0:
                        _tap(nc, "dcp00", dcp[:, :])
                        _tap(nc, "rec200", rec2[:, :])
                # restore natural feature order (partition remap via DMA)
                for j in range(2):
                    r0 = base + 32 * j
                    nc.sync.dma_start(
                        out=OTpre[tK][r0:r0 + 32, :], in_=opl[64 * j:64 * j + 32, :])
        for t in range(2):
            nc.vector.tensor_add(OT[t][:, :], OTpre[t][:, :], _f(QT[t][:, :]))
        _tap(nc, "OTpre0", OTpre[0][:, :])
        _tap(nc, "OT0", OT[0][:, :])

        # ---------------- LN1 ----------------
        OnT = [zn_pool.tile([128, N], F32R, tag="zn", name=f"OnT{k}")
               for k in range(2)]
        _layernorm_featmajor(nc, tc, work, cones, eps_ap, OT, OnT, DV, "ln1")

        # ---------------- FFN ----------------
        with tc.tile_pool(name="ps_ffn", bufs=2, space="PSUM") as ps_ffn:
            HT = []
            for m in range(4):
                hps = ps_ffn.tile([128, N], F32, tag="ffps", name="hps")
                for k in range(2):
                    for c in range(NCH):
                        nc.tensor.matmul(
                            hps[:, c * CH:(c + 1) * CH],
                            W1_sb[k][:, m * 128:(m + 1) * 128],
                            OnT[k][:, c * CH:(c + 1) * CH],
                            start=(k == 0), stop=(k == 1),
                        )
                ht = xy_pool.tile([128, N], F32R, tag="xy", name=f"HT{m}")
                nc.scalar.activation(ht[:, :], hps[:, :], AF.Gelu,
                                     bias=b1_sb[:, m:m + 1])
                HT.append(ht)

            for m in range(2):
                fps = ps_ffn.tile([128, N], F32, tag="ffps", name="fps")
                for k in range(4):
                    for c in range(NCH):
                        nc.tensor.matmul(
                            fps[:, c * CH:(c + 1) * CH],
                            W2_sb[k][:, m * 128:(m + 1) * 128],
                            HT[k][:, c * CH:(c + 1) * CH],
                            start=(k == 0), stop=(k == 3),
                        )
                fin = zn_pool.tile([128, N], F32, tag="zn", name=f"finT{m}")
                nc.vector.scalar_tensor_tensor(
                    fin[:, :], fps[:, :], b2_sb[:, m:m + 1], _f(OT[m][:, :]),
                    op0=ALU.add, op1=ALU.add,
                )
                nc.sync.dma_start(out=outT_d[m * 128:(m + 1) * 128, :], in_=fin[:, :])


def build_nc():
    nc = bacc.Bacc("TRN2", target_bir_lowering=False, debug=False)
    with tile.TileContext(nc) as tc:
        build_device_kernel(nc, tc)
    nc.compile()
    return nc


def _prep_in_maps(inputs):
    gi = lambda k: np.asarray(inputs[k], dtype=np.float32)
    x, y = gi("x"), gi("y")
    Wq, Wk, Wv = gi("Wq"), gi("Wk"), gi("Wv")
    W1, W2 = gi("W1"), gi("W2")
    bq, bk, bv, b1, b2 = gi("bq"), gi("bk"), gi("bv"), gi("b1"), gi("b2")
    g0, b0 = gi("ln0_g"), gi("ln0_b")
    g1, b1n = gi("ln1_g"), gi("ln1_b")

    xT = np.ascontiguousarray(x.transpose(0, 2, 1))
    yT = np.ascontiguousarray(y.transpose(0, 2, 1))
    Wq_eff = np.ascontiguousarray(Wq * g0[:, None])
    bq_eff = np.ascontiguousarray(bq + b0 @ Wq)
    W1_eff = np.ascontiguousarray(W1 * g1[:, None])
    b1_eff = np.ascontiguousarray(b1 + b1n @ W1)

    shared = dict(Wq=Wq_eff, Wk=Wk, Wv=Wv, W1=W1_eff, W2=W2,
                  bq=bq_eff, bk=bk, bv=bv.reshape(1, DV), b1=b1_eff, b2=b2,
                  cones=np.ones((128, 128), np.float32),
                  conesb=np.ones((128, 32), ml_dtypes.bfloat16))
    return [dict(xT=xT[b], yT=yT[b], **shared) for b in range(B)]


def kernel(**inputs) -> np.ndarray:
    if "nc" not in _NC_CACHE:
        _NC_CACHE["nc"] = build_nc()
    nc = _NC_CACHE["nc"]
    in_maps = _prep_in_maps(inputs)
    res = run_bass_kernel_spmd(nc, in_maps, core_ids=list(range(B)))
    out = np.stack([res.results[b]["outT"].T for b in range(B)])
    return np.ascontiguousarray(out.astype(np.float32))
